# revision 1
# baseline (speedup 1.0000x reference)
"""HEALPix padding (p=2) kernel for Trainium2 (Bass/Tile).

Input : data (96, 256, 64, 64) f32 = (B*12 faces, C, H, W), B=8, plus scalar p=2.
Output: (96, 256, 68, 68) f32.

Sharding: data-parallel over the batch dim. Each of the 8 NeuronCores gets one
group of 12 HEALPix faces (12, 256, 64, 64) so every cross-face halo gather is
core-local.

Per-core plan (per 128-channel chunk, channels on SBUF partitions):
  - Two staging DMAs bring the first-2 / last-2 rows of every face into SBUF.
  - Face tiles stream through SBUF (contiguous 2MB loads). On arrival the
    tile's first-2/last-2 columns are extracted on-chip (column strips are
    non-contiguous in DRAM, so DMAing them directly would be descriptor-bound)
    and its interior is copied into the padded 68x68 plane.
  - Once a face's neighbor column strips are all staged, its halo strips and
    corners are assembled from the staged edges and the finished plane is
    stored with one contiguous 2.3MB DMA.
The face load order is chosen so the column-strip producer of each face lands
before its consumers while keeping at most ~5 padded planes live.
"""

import numpy as np

_FACES = 12
_PAD = 2

# Load order: keeps peak live planes ~5 while satisfying column-strip deps.
_ORDER = [1, 2, 6, 0, 5, 3, 7, 4, 9, 10, 11, 8]


def _col_deps(g):
    """Faces whose column strips face g's halo assembly reads."""
    if g < 4:  # _pn
        return ((g + 1) % 4, 4 + (g + 1) % 4)
    if g < 8:  # _pe
        i = g - 4
        return (i, (i + 3) % 4, 8 + i)
    i = g - 8  # _ps
    return (4 + i, 8 + (i + 3) % 4)


def _assemble(nc, g, pl, colL, colR, toprows, botrows, H, OH):
    """Emit halo strip + corner ops for face g into plane `pl`.

    colL[f]/colR[f]: (P, H, 2) staged first/last-2 columns of face f.
    toprows/botrows: (P, 12, 2, W) staged first/last-2 rows of all faces.
    """
    W = H
    V = nc.vector
    p2 = pl.rearrange("p a b -> p (a b)")
    tr_f = toprows.rearrange("p f r w -> p (f r w)")
    br_f = botrows.rearrange("p f r w -> p (f r w)")

    if g < 4:  # _pn
        i = g
        t = (i + 1) % 4
        tl = (i + 2) % 4
        l = (i + 3) % 4
        bl = l
        b = 4 + i
        br = 8 + i
        r = 4 + (i + 1) % 4
        tr = t
        # top[r_, c] = t[c, 1 - r_]   (rot90 of t's first-2 cols)
        for r_ in range(2):
            V.tensor_copy(pl[:, r_, 2:2 + W], colL[t][:, :, 1 - r_])
        # left[i_, j] = l[1 - j, i_]  (rot90 of l's first-2 rows)
        for j in range(2):
            V.tensor_copy(pl[:, 2:2 + H, j], toprows[:, l, 1 - j, :])
        V.tensor_copy(pl[:, H + 2:H + 4, 2:2 + W], toprows[:, b, :, :])
        V.tensor_copy(pl[:, 2:2 + H, W + 2:W + 4], colL[r][:])
        # tl corner = rot180(tl_face[0:2, 0:2])
        for i_ in range(2):
            for j_ in range(2):
                V.tensor_copy(pl[:, i_:i_ + 1, j_:j_ + 1],
                       toprows[:, tl, 1 - i_:2 - i_, 1 - j_:2 - j_])
        V.tensor_copy(pl[:, H + 2:H + 4, 0:2], toprows[:, bl, :, W - 2:W])
        V.tensor_copy(pl[:, 0:2, W + 2:W + 4], botrows[:, tr, :, 0:2])
        V.tensor_copy(pl[:, H + 2:H + 4, W + 2:W + 4], toprows[:, br, :, 0:2])

    elif g < 8:  # _pe
        i = g - 4
        t = i
        l = (i + 3) % 4
        bl = 4 + (i + 3) % 4
        b = 8 + (i + 3) % 4
        r = 8 + i
        tr = 4 + (i + 1) % 4
        V.tensor_copy(pl[:, 0:2, 2:2 + W], botrows[:, t, :, :])
        V.tensor_copy(pl[:, 2:2 + H, 0:2], colR[l][:])
        V.tensor_copy(pl[:, H + 2:H + 4, 2:2 + W], toprows[:, b, :, :])
        V.tensor_copy(pl[:, 2:2 + H, W + 2:W + 4], colL[r][:])
        # tl corner (computed): [[.5(t[H-2,0]+l[0,W-2]), t[H-2,0]],
        #                        [l[0,W-2], .5(t[H-1,0]+l[0,W-1])]]
        V.tensor_copy(pl[:, 0:1, 1:2], colL[t][:, H - 2:H - 1, 0:1])
        V.tensor_copy(pl[:, 1:2, 0:1], toprows[:, l, 0:1, W - 2:W - 1])
        d = p2[:, 0:OH + 2:OH + 1]
        V.tensor_add(d, colL[t].rearrange("p a b -> p (a b)")[:, 2 * (H - 2):2 * H:2],
                     tr_f[:, l * 2 * W + W - 2:l * 2 * W + W])
        V.tensor_scalar_mul(d, d, 0.5)
        # br corner (computed): [[.5(b[0,W-1]+r[H-1,0]), r[H-1,1]],
        #                        [b[1,W-1], .5(b[1,W-1]+r[H-1,1])]]
        V.tensor_copy(pl[:, H + 2:H + 3, W + 3:W + 4], botrows[:, r, 1:2, 1:2])
        V.tensor_copy(pl[:, H + 3:H + 4, W + 2:W + 3], toprows[:, b, 1:2, W - 1:W])
        st = (H + 2) * OH + (W + 2)
        d = p2[:, st:st + OH + 2:OH + 1]
        V.tensor_add(d, tr_f[:, b * 2 * W + W - 1:b * 2 * W + 2 * W:W],
                     br_f[:, r * 2 * W + W:r * 2 * W + W + 2])
        V.tensor_scalar_mul(d, d, 0.5)
        V.tensor_copy(pl[:, H + 2:H + 4, 0:2], toprows[:, bl, :, W - 2:W])
        V.tensor_copy(pl[:, 0:2, W + 2:W + 4], botrows[:, tr, :, 0:2])

    else:  # _ps
        i = g - 8
        t = 4 + (i + 1) % 4
        tl = i
        l = 4 + i
        bl = 8 + (i + 3) % 4
        b = bl
        br = 8 + (i + 2) % 4
        r = 8 + (i + 1) % 4
        tr = r
        V.tensor_copy(pl[:, 0:2, 2:2 + W], botrows[:, t, :, :])
        V.tensor_copy(pl[:, 2:2 + H, 0:2], colR[l][:])
        # bottom[r_, c] = b[c, W-1-r_]  (rot90 of b's last-2 cols)
        for r_ in range(2):
            V.tensor_copy(pl[:, H + 2 + r_, 2:2 + W], colR[b][:, :, 1 - r_])
        # right[i_, j] = r[H-1-j, i_]   (rot90 of r's last-2 rows)
        for j in range(2):
            V.tensor_copy(pl[:, 2:2 + H, W + 2 + j], botrows[:, r, 1 - j, :])
        V.tensor_copy(pl[:, 0:2, 0:2], botrows[:, tl, :, W - 2:W])
        V.tensor_copy(pl[:, H + 2:H + 4, 0:2], toprows[:, bl, :, W - 2:W])
        V.tensor_copy(pl[:, 0:2, W + 2:W + 4], botrows[:, tr, :, 0:2])
        # br corner = rot180(br_face[H-2:H, W-2:W])
        for i_ in range(2):
            for j_ in range(2):
                V.tensor_copy(pl[:, H + 2 + i_:H + 3 + i_, W + 2 + j_:W + 3 + j_],
                       botrows[:, br, 1 - i_:2 - i_, W - 1 - j_:W - j_])


def _build_nc(C=256, H=64, PCHUNK=128, tiles_bufs=2, planes_bufs=6):
    import concourse.bass as bass
    import concourse.mybir as mybir
    import concourse.tile_scheduler as _ts
    import concourse.tile_sem_assignment as _tsa
    from concourse.tile import TileContext

    # All HWDGE DMAs here issue from the SP engine (one FIFO ring), so one
    # completion-tracking lane is both sufficient and tighter: with 8
    # round-robin lanes, slot-reuse deps span two DMAHW sems and the DMA
    # instruction exceeds walrus's sync-wait slot limit ("Too many sync wait
    # commands" in CoreV2Gen setupSyncWait).
    _ts.NUM_HWDGE_SEMS = 1
    _tsa.NUM_HWDGE_SEMS = 1

    f32 = mybir.dt.float32
    W = H
    OH = H + 2 * _PAD
    nc = bass.Bass()
    x = nc.dram_tensor("data", (_FACES, C, H, W), f32, kind="ExternalInput")
    y = nc.dram_tensor("out", (_FACES, C, OH, OH), f32, kind="ExternalOutput")

    with TileContext(nc) as tc:
        with (
            tc.tile_pool(name="tiles", bufs=tiles_bufs) as tpool,
            tc.tile_pool(name="planes", bufs=planes_bufs) as ppool,
            tc.tile_pool(name="rows", bufs=4) as rpool,
            tc.tile_pool(name="cols", bufs=26) as cpool,
        ):
            for c0 in range(0, C, PCHUNK):
                P = PCHUNK
                cs = slice(c0, c0 + P)
                toprows = rpool.tile([P, _FACES, 2, W], f32,
                                     name=f"toprows_{c0}", tag="rows")
                botrows = rpool.tile([P, _FACES, 2, W], f32,
                                     name=f"botrows_{c0}", tag="rows")
                nc.sync.dma_start(out=toprows[:],
                                    in_=x[:, cs, 0:2, :].transpose((1, 0, 2, 3)))
                nc.sync.dma_start(out=botrows[:],
                                    in_=x[:, cs, H - 2:H, :].transpose((1, 0, 2, 3)))

                colL, colR, planes = {}, {}, {}
                loaded, assembled = set(), set()
                for f in _ORDER:
                    tile = tpool.tile([P, H, W], f32,
                                      name=f"tile_{c0}_{f}", tag="tile")
                    nc.sync.dma_start(
                        out=tile.rearrange("p a b -> p (a b)"),
                        in_=x[f, cs].rearrange("c a b -> c (a b)"))
                    cl = cpool.tile([P, H, 2], f32, name=f"colL_{c0}_{f}", tag="col")
                    cr = cpool.tile([P, H, 2], f32, name=f"colR_{c0}_{f}", tag="col")
                    nc.vector.tensor_copy(cl[:], tile[:, :, 0:2])
                    nc.vector.tensor_copy(cr[:], tile[:, :, W - 2:W])
                    colL[f], colR[f] = cl, cr
                    pl = ppool.tile([P, OH, OH], f32,
                                    name=f"plane_{c0}_{f}", tag="plane")
                    nc.vector.tensor_copy(pl[:, 2:2 + H, 2:2 + W], tile[:])
                    planes[f] = pl
                    loaded.add(f)
                    for g in _ORDER:
                        if g in assembled or g not in loaded:
                            continue
                        if all(d in loaded for d in _col_deps(g)):
                            _assemble(nc, g, planes[g], colL, colR,
                                      toprows, botrows, H, OH)
                            nc.sync.dma_start(
                                out=y[g, cs].rearrange("c a b -> c (a b)"),
                                in_=planes[g].rearrange("p a b -> p (a b)"))
                            assembled.add(g)
                assert len(assembled) == _FACES

    # walrus's DMA_DIRECT2D lowering accepts a single sync-wait slot, but
    # slot-reuse deps give some DMAs two (compute sem + DMAHW sem). Every DMA
    # here issues from the SP sequencer in program order onto one HWDGE ring
    # (qSPDynamicHW), and per-ring full-completion order equals issue order,
    # so DMA-vs-DMA semaphore waits are redundant: drop them, keeping the
    # compute-engine wait.
    import concourse.mybir as mybir
    max_dve_wait_on_dma = 0
    for blk in nc.m.functions[0].blocks:
        for inst in blk.instructions:
            if not isinstance(inst, mybir.InstDMACopy):
                continue
            assert inst.engine == mybir.EngineType.SP, inst.concise()
            si = inst.sync_info
            if si is None:
                continue
            for w in si.on_wait:
                if w.ant_name.startswith("DVE"):
                    max_dve_wait_on_dma = max(max_dve_wait_on_dma, w.wait_value)
            if len(si.on_wait) <= 1:
                continue
            keep = [w for w in si.on_wait if not w.ant_name.startswith("DMAHW")]
            if not keep:
                keep = [max(si.on_wait, key=lambda w: w.wait_value)]
            assert len(keep) == 1, [w.ant_name for w in si.on_wait]
            si.on_wait = keep
            inst.sync_info = si

    # The SP kernel-tail Drain waits on [DVE_total, DMAHW0_total]; the final
    # store DMA already waits on the same DVE total and the DMAHW0 wait
    # covers that store's completion, so the DVE wait is transitively
    # implied — drop it to fit the 1-wait slot.
    for blk in nc.m.functions[0].blocks:
        for inst in blk.instructions:
            si = inst.sync_info
            if si is None or len(si.on_wait) <= 1:
                continue
            assert isinstance(inst, mybir.InstDrain), inst.concise()
            dve = [w for w in si.on_wait if w.ant_name.startswith("DVE")]
            dma = [w for w in si.on_wait if w.ant_name.startswith("DMAHW")]
            assert len(dve) == 1 and len(dma) == 1, inst.concise()
            assert dve[0].wait_value <= max_dve_wait_on_dma, inst.concise()
            si.on_wait = dma
            inst.sync_info = si

    nc.finalize()
    return nc


_NC_CACHE = {}


def _get_nc():
    if "nc" not in _NC_CACHE:
        _NC_CACHE["nc"] = _build_nc()
    return _NC_CACHE["nc"]


def _run(data, **kwargs):
    from concourse import bass_utils

    data = np.ascontiguousarray(np.asarray(data, dtype=np.float32))
    n_cores = 8
    group = data.shape[0] // n_cores
    assert group == _FACES
    nc = _get_nc()
    in_maps = [{"data": data[g * group:(g + 1) * group]} for g in range(n_cores)]
    return bass_utils.run_bass_kernel_spmd(
        nc, in_maps, core_ids=list(range(n_cores)), **kwargs)


def kernel(data, p):
    assert int(p) == _PAD
    res = _run(data)
    return np.concatenate([r["out"] for r in res.results], axis=0)

